# revision 4
# baseline (speedup 1.0000x reference)
"""Trainium2 Bass kernel for attention energies + softmax.

Computes: energies = encoder_outputs[8192,4096] @ hidden[4096] ; softmax -> [1,1,8192]

Sharding: encoder_outputs split along seq_len across 8 NeuronCores
(1024 rows each). Each core streams its 16 MiB shard from HBM and
computes local energies with a fused DVE multiply+accumulate
(scalar_tensor_tensor). Cross-core coupling is only the softmax
normalizer: each core computes local (max, sum_exp) stats, AllGathers
the 8 stat pairs (64 B), finalizes softmax for its own rows, and
writes its 1024-row output shard. The host concatenates the shards.
"""

from contextlib import ExitStack

import numpy as np

import concourse.bacc as bacc
import concourse.tile as tile
from concourse import masks, mybir
from concourse.bass_utils import run_bass_kernel_spmd

P = 128          # SBUF partitions
H = 4096         # hidden dim
S = 8192         # full seq len
NCORES = 8
SL = S // NCORES  # 1024 rows per core
T = SL // P       # 8 seq tiles per core
MM_N = 512        # fp32 matmul moving-operand max

F32 = mybir.dt.float32
AX = mybir.AxisListType
OP = mybir.AluOpType
ACT = mybir.ActivationFunctionType


def build_kernel():
    nc = bacc.Bacc(
        "TRN2",
        target_bir_lowering=False,
        debug=False,
        num_devices=NCORES,
    )
    hidden_d = nc.dram_tensor("hidden", [1, H], F32, kind="ExternalInput").ap()
    eo_d = nc.dram_tensor("eo", [SL, H], F32, kind="ExternalInput").ap()
    out_d = nc.dram_tensor("out", [T, P], F32, kind="ExternalOutput").ap()

    with tile.TileContext(nc) as tc, ExitStack() as ctx:
        singles = ctx.enter_context(tc.tile_pool(name="singles", bufs=1))
        tiles = ctx.enter_context(tc.tile_pool(name="tiles", bufs=3))
        scratch = ctx.enter_context(tc.tile_pool(name="scratch", bufs=2))
        psum = ctx.enter_context(tc.tile_pool(name="psum", bufs=1, space="PSUM"))
        psum2 = ctx.enter_context(tc.tile_pool(name="psum2", bufs=2, space="PSUM"))
        dram = ctx.enter_context(tc.tile_pool(name="dram", bufs=1, space="DRAM"))

        # ---- constants ----
        ident = singles.tile([P, P], F32)
        masks.make_identity(nc, ident[:])
        ones_col = singles.tile([P, 1], F32)
        nc.vector.memset(ones_col[:], 1.0)
        ones_row = singles.tile([1, P], F32)
        nc.vector.memset(ones_row[:], 1.0)

        # ---- hidden: small DMA + PE broadcast to all 128 partitions ----
        h_row = singles.tile([1, H], F32)
        nc.sync.dma_start(out=h_row[:], in_=hidden_d)
        h_sb = singles.tile([P, H], F32)
        for j in range(0, H, MM_N):
            hb_ps = psum2.tile([P, MM_N], F32)
            nc.tensor.matmul(hb_ps[:], ones_row[:], h_row[:, j : j + MM_N])
            nc.scalar.copy(h_sb[:, j : j + MM_N], hb_ps[:])

        # ---- local energies: e[p, t] = dot(eo[t*128+p, :], hidden) ----
        eo_t = eo_d.rearrange("(t p) h -> t p h", p=P)
        e_sb = singles.tile([P, T], F32)
        for t in range(T):
            x = tiles.tile([P, H], F32)
            nc.sync.dma_start(out=x[:], in_=eo_t[t])
            prod = scratch.tile([P, H], F32)
            nc.vector.scalar_tensor_tensor(
                out=prod[:],
                in0=x[:],
                scalar=1.0,
                in1=h_sb[:],
                op0=OP.mult,
                op1=OP.mult,
                accum_out=e_sb[:, t : t + 1],
            )

        # ---- local stats: nm = -max(e_local), s = sum(exp(e_local - max)) ----
        stats_sb = singles.tile([1, 2], F32)  # [nm, s]
        m1 = singles.tile([P, 1], F32)
        nc.vector.tensor_reduce(out=m1[:], in_=e_sb[:], axis=AX.X, op=OP.max)
        m1t_ps = psum.tile([1, P], F32)
        nc.tensor.transpose(m1t_ps[:], m1[:], ident[:])
        nc.vector.tensor_reduce(
            out=stats_sb[:, 0:1], in_=m1t_ps[:], axis=AX.X, op=OP.max, negate=True
        )
        nmb_ps = psum.tile([P, 1], F32)
        nc.tensor.matmul(nmb_ps[:], ones_row[:], stats_sb[:, 0:1])
        nmb_sb = singles.tile([P, 1], F32)
        nc.scalar.copy(nmb_sb[:], nmb_ps[:])
        expl = singles.tile([P, T], F32)
        srow = singles.tile([P, 1], F32)
        nc.scalar.activation(
            expl[:], e_sb[:], ACT.Exp, bias=nmb_sb[:], scale=1.0, accum_out=srow[:]
        )
        s_ps = psum.tile([1, 1], F32)
        nc.tensor.matmul(s_ps[:], srow[:], ones_col[:])
        nc.vector.tensor_copy(stats_sb[:, 1:2], s_ps[:])

        # ---- AllGather the 8 stat pairs (64 B) ----
        cc_in = dram.tile([1, 2], F32)
        cc_out = dram.tile([NCORES, 2], F32)
        nc.sync.dma_start(out=cc_in[:], in_=stats_sb[:])
        nc.gpsimd.collective_compute(
            "AllGather",
            OP.bypass,
            replica_groups=[list(range(NCORES))],
            ins=[cc_in[:].opt()],
            outs=[cc_out[:].opt()],
        )
        st = singles.tile([1, NCORES, 2], F32)
        nc.sync.dma_start(out=st[:], in_=cc_out[:])

        # ---- global stats: -M = min_r nm_r ; S = sum_r s_r * exp(m_r - M) ----
        scal2 = singles.tile([1, 2], F32)  # [-M, 1/S]
        nc.vector.tensor_reduce(
            out=scal2[:, 0:1], in_=st[:, :, 0], axis=AX.X, op=OP.min
        )
        w = singles.tile([1, NCORES], F32)
        # exp(m_r - M) = exp(-1 * nm_r + (-M))
        nc.scalar.activation(
            w[:], st[:, :, 0], ACT.Exp, bias=scal2[:, 0:1], scale=-1.0
        )
        w2 = singles.tile([1, NCORES], F32)
        nc.vector.tensor_tensor(out=w2[:], in0=w[:], in1=st[:, :, 1], op=OP.mult)
        S_sc = singles.tile([1, 1], F32)
        nc.vector.tensor_reduce(out=S_sc[:], in_=w2[:], axis=AX.X, op=OP.add)
        nc.vector.reciprocal(scal2[:, 1:2], S_sc[:])

        # ---- finalize: out = exp(e - M) / S for the local shard ----
        bc_ps = psum.tile([P, 2], F32)
        nc.tensor.matmul(bc_ps[:], ones_row[:], scal2[:])
        bc_sb = singles.tile([P, 2], F32)
        nc.scalar.copy(bc_sb[:], bc_ps[:])
        q = singles.tile([P, T], F32)
        nc.scalar.activation(
            q[:], e_sb[:], ACT.Exp, bias=bc_sb[:, 0:1], scale=1.0
        )
        o = singles.tile([P, T], F32)
        nc.vector.tensor_scalar_mul(o[:], q[:], bc_sb[:, 1:2])
        o_t_ps = psum.tile([T, P], F32)
        nc.tensor.transpose(o_t_ps[:], o[:], ident[:])
        o_t_sb = singles.tile([T, P], F32)
        nc.scalar.copy(o_t_sb[:], o_t_ps[:])
        nc.sync.dma_start(out=out_d, in_=o_t_sb[:])

    nc.compile()
    return nc


_NC = None


def _get_nc():
    global _NC
    if _NC is None:
        _NC = build_kernel()
    return _NC


def _make_in_maps(hidden: np.ndarray, encoder_outputs: np.ndarray):
    hidden = np.ascontiguousarray(np.asarray(hidden, dtype=np.float32)).reshape(1, H)
    eo = np.ascontiguousarray(np.asarray(encoder_outputs, dtype=np.float32))
    assert eo.shape == (S, H), eo.shape
    return [
        {"hidden": hidden, "eo": eo[c * SL : (c + 1) * SL]} for c in range(NCORES)
    ]


def kernel(hidden: np.ndarray, encoder_outputs: np.ndarray) -> np.ndarray:
    nc = _get_nc()
    in_maps = _make_in_maps(hidden, encoder_outputs)
    res = run_bass_kernel_spmd(nc, in_maps, core_ids=list(range(NCORES)))
    parts = [
        np.asarray(res.results[c]["out"], dtype=np.float32).reshape(SL)
        for c in range(NCORES)
    ]
    return np.concatenate(parts).reshape(1, 1, S)


if __name__ == "__main__":
    rng = np.random.default_rng(0)
    h = rng.standard_normal((1, H), dtype=np.float32)
    eo = rng.standard_normal((S, H), dtype=np.float32)
    got = kernel(hidden=h, encoder_outputs=eo)
    e = eo.astype(np.float64) @ h.reshape(-1).astype(np.float64)
    e -= e.max()
    p = np.exp(e)
    want = (p / p.sum()).reshape(1, 1, S)
    err = np.abs(got.astype(np.float64) - want)
    rel = err.max() / np.abs(want).max()
    print("max abs err:", err.max(), "rel:", rel)
